# revision 1
# baseline (speedup 1.0000x reference)
"""Trainium2 Bass kernel for nn_DeformAtten1D (deformable 1D attention).

Self-contained: takes the FULL unsharded inputs as numpy arrays, shards
batch-parallel across 8 NeuronCores, runs a Bass/Tile kernel per core, and
reassembles the full [B, L, C] float32 output.
"""
import sys
for _p in ('/opt/trn_rl_repo', '/root/.axon_site/_ro/trn_rl_repo'):
    if _p not in sys.path:
        sys.path.insert(0, _p)

import numpy as np
import ml_dtypes

import concourse.bass as bass
import concourse.bacc as bacc
import concourse.mybir as mybir
import concourse.tile as tile
from concourse.masks import make_identity
from concourse.library_config import mlp

F32 = mybir.dt.float32
BF16 = mybir.dt.bfloat16
I32 = mybir.dt.int32
I16 = mybir.dt.int16
AF = mybir.ActivationFunctionType
OP = mybir.AluOpType
BF = ml_dtypes.bfloat16


class Cfg:
    def __init__(self, B_SH, L, C, H, G, K=5):
        self.B_SH, self.L, self.C, self.H, self.G, self.K = B_SH, L, C, H, G, K
        self.GC = C // G
        self.DH = C // H
        assert self.DH == 64
        self.PAD = K // 2
        self.L4 = L + 2 * self.PAD
        self.sn = L / (self.L4 - 1)
        self.KT = C // 128
        self.NS = L // 128
        self.SLG = self.GC // 128
        self.NB = L // 512
        self.NO = C // 512
        self.MH = self.KT // self.NO   # m-tiles per weight half-block
        self.GR = 12
        self.GO = 6
        self.P2 = H // 2
        assert self.P2 == self.KT
        assert L % 512 == 0 and C % 512 == 0 and self.GC % 128 == 0


def declare(nc, cfg):
    c = cfg
    t = {}
    t['xgbf'] = nc.dram_tensor("xgbf", [c.B_SH * (c.L + c.GR), c.C], BF16, kind="ExternalInput")
    t['rtab'] = nc.dram_tensor("rtab", [c.C, c.L], BF16, kind="ExternalInput")
    t['w1t'] = nc.dram_tensor("w1t", [c.GC, c.K, c.GC], BF16, kind="ExternalInput")
    t['w2c'] = nc.dram_tensor("w2c", [c.GC, 1], BF16, kind="ExternalInput")
    for nm in ('wqT', 'wkT', 'wvT', 'woutT'):
        t[nm] = nc.dram_tensor(nm, [c.C, c.C], BF16, kind="ExternalInput")
    t['bq_col'] = nc.dram_tensor("bq_col", [c.C, 1], F32, kind="ExternalInput")
    t['bq_row'] = nc.dram_tensor("bq_row", [1, c.C], BF16, kind="ExternalInput")
    t['bk_row'] = nc.dram_tensor("bk_row", [1, c.C], BF16, kind="ExternalInput")
    t['boff1_col'] = nc.dram_tensor("boff1_col", [c.GC, 1], F32, kind="ExternalInput")
    t['boff2c'] = nc.dram_tensor("boff2c", [1, 1], F32, kind="ExternalInput")
    t['bout_col'] = nc.dram_tensor("bout_col", [c.C, 1], F32, kind="ExternalInput")
    t['posg'] = nc.dram_tensor("posg", [128, c.NS], F32, kind="ExternalInput")
    t['out2d'] = nc.dram_tensor("out2d", [c.B_SH * c.L, c.C], F32, kind="ExternalOutput")
    dbg = getattr(c, 'debug', False)
    xs_kind = "ExternalOutput" if dbg else "Internal"
    t['xs_scr'] = nc.dram_tensor("xs_scr", [c.B_SH * c.L, c.C], BF16, kind=xs_kind)
    t['qt_scr'] = nc.dram_tensor("qt_scr", [c.NS * 128, c.C], BF16,
                                 kind=("ExternalOutput" if dbg else "Internal"))
    if dbg:
        t['d_gq'] = nc.dram_tensor("d_gq", [128, c.KT, c.L + 4], BF16, kind="ExternalOutput")
        t['d_psm'] = nc.dram_tensor("d_psm", [c.G, 128, c.NS], F32, kind="ExternalOutput")
        t['d_w1'] = nc.dram_tensor("d_w1", [c.G, 128, c.NS], F32, kind="ExternalOutput")
        t['d_h'] = nc.dram_tensor("d_h", [c.G, 128, c.SLG, c.L + 2], BF16, kind="ExternalOutput")
        t['d_kT'] = nc.dram_tensor("d_kT", [128, c.NS, c.C], BF16, kind="ExternalOutput")
        t['d_ablk'] = nc.dram_tensor("d_ablk", [c.P2, 128, 128], F32, kind="ExternalOutput")
        t['d_v'] = nc.dram_tensor("d_v", [128, c.KT, c.L], BF16, kind="ExternalOutput")
        t['d_idx'] = nc.dram_tensor("d_idx", [c.G, 128, 2 * c.NS], I16, kind="ExternalOutput")
        t['d_wrap'] = nc.dram_tensor("d_wrap", [c.G, 128, 16 * c.NS], I16, kind="ExternalOutput")
        t['d_g01'] = nc.dram_tensor("d_g01", [c.G, 128, 2 * c.NS, c.GC], BF16, kind="ExternalOutput")
        t['d_ao'] = nc.dram_tensor("d_ao", [128, c.KT, c.L], BF16, kind="ExternalOutput")
    return t


def build(tc, t, cfg, ctx):
    c = cfg
    nc = tc.nc
    L, C, KT, NS, NB, NO, GC, SLG, G, K, MH = (c.L, c.C, c.KT, c.NS, c.NB, c.NO,
                                               c.GC, c.SLG, c.G, c.K, c.MH)
    scale = C ** -0.5

    nc.gpsimd.load_library(mlp)

    konst = ctx.enter_context(tc.tile_pool(name="konst", bufs=1))
    big = ctx.enter_context(tc.tile_pool(name="big", bufs=1))
    wp = ctx.enter_context(tc.tile_pool(name="wp", bufs=2))
    med = ctx.enter_context(tc.tile_pool(name="med", bufs=2))
    hp = ctx.enter_context(tc.tile_pool(name="hp", bufs=1))
    sm = ctx.enter_context(tc.tile_pool(name="sm", bufs=2))
    rp = ctx.enter_context(tc.tile_pool(name="rp", bufs=2))
    yp = ctx.enter_context(tc.tile_pool(name="yp", bufs=3))
    qp = ctx.enter_context(tc.tile_pool(name="qp", bufs=2))
    dscr = ctx.enter_context(tc.tile_pool(name="dscr", bufs=2, space="DRAM"))
    psmm = ctx.enter_context(tc.tile_pool(name="psmm", bufs=4, space="PSUM"))
    pssc = ctx.enter_context(tc.tile_pool(name="pssc", bufs=2, space="PSUM"))
    pstr = ctx.enter_context(tc.tile_pool(name="pstr", bufs=1, space="PSUM"))
    psoff = ctx.enter_context(tc.tile_pool(name="psoff", bufs=1, space="PSUM"))

    # ---- constants ----
    w1t_sb = konst.tile([128, SLG, K, GC], BF16, tag="w1t")
    nc.sync.dma_start(out=w1t_sb[:], in_=bass.AP(
        tensor=t['w1t'].ap().tensor, offset=0,
        ap=[[K * GC, 128], [128 * K * GC, SLG], [GC, K], [1, GC]]))
    w2_sb = konst.tile([128, SLG], BF16, tag="w2")
    nc.sync.dma_start(out=w2_sb[:], in_=bass.AP(
        tensor=t['w2c'].ap().tensor, offset=0, ap=[[1, 128], [128, SLG]]))
    posg_sb = konst.tile([128, NS], F32, tag="posg")
    nc.sync.dma_start(out=posg_sb[:], in_=t['posg'].ap())
    bq_col_sb = konst.tile([128, KT], F32, tag="bqc")
    nc.sync.dma_start(out=bq_col_sb[:], in_=bass.AP(
        tensor=t['bq_col'].ap().tensor, offset=0, ap=[[1, 128], [128, KT]]))
    boff1_sb = konst.tile([128, SLG], F32, tag="bo1")
    nc.sync.dma_start(out=boff1_sb[:], in_=bass.AP(
        tensor=t['boff1_col'].ap().tensor, offset=0, ap=[[1, 128], [128, SLG]]))
    boff2_sb = konst.tile([128, 1], F32, tag="bo2")
    nc.sync.dma_start(out=boff2_sb[:], in_=bass.AP(
        tensor=t['boff2c'].ap().tensor, offset=0, ap=[[0, 128], [1, 1]]))
    bout_sb = konst.tile([128, KT], F32, tag="boc")
    nc.sync.dma_start(out=bout_sb[:], in_=bass.AP(
        tensor=t['bout_col'].ap().tensor, offset=0, ap=[[1, 128], [128, KT]]))
    bqr_sb = konst.tile([1, C], BF16, tag="bqr")
    nc.sync.dma_start(out=bqr_sb[:], in_=t['bq_row'].ap())
    bkr_sb = konst.tile([1, C], BF16, tag="bkr")
    nc.sync.dma_start(out=bkr_sb[:], in_=t['bk_row'].ap())
    ones_sb = konst.tile([1, 128], BF16, tag="ones")
    nc.vector.memset(ones_sb[:], 1.0)
    ident = konst.tile([128, 128], F32, tag="ident")
    make_identity(nc, ident[:])

    DBG = getattr(c, 'debug', False)
    xg = t['xgbf'].ap()
    LG = L + c.GR

    def load_whalf(wname, hi):
        wh = wp.tile([128, KT, 512], BF16, tag="wblk")
        nc.sync.dma_start(out=wh[:], in_=bass.AP(
            tensor=t[wname].ap().tensor, offset=512 * hi,
            ap=[[C, 128], [128 * C, KT], [1, 512]]))
        return wh

    for b in range(c.B_SH):
        # ================= phase A: xT load, q-pass, qT-pass =================
        xT = big.tile([128, KT, L], BF16, tag="bigX")
        for kt in range(KT):
            nc.sync.dma_start(
                out=xT[:, kt, :],
                in_=xg[b * LG + c.GO: b * LG + c.GO + L,
                       128 * kt:128 * (kt + 1)].rearrange("l c -> c l"))

        gq = big.tile([128, KT, L + 4], BF16, tag="bigGV")
        nc.gpsimd.memset(gq[:, :, 0:2], 0.0)
        nc.gpsimd.memset(gq[:, :, L + 2:L + 4], 0.0)
        qt_ap = t['qt_scr'].ap()
        for hi in range(NO):
            wq_h = load_whalf('wqT', hi)
            # q-pass -> gq padded slabs (gq[o, l], o = m*128+p); half hi covers m in [hi*MH, ...)
            for mm_ in range(MH):
                m = hi * MH + mm_
                for n in range(NB):
                    ps = psmm.tile([128, 512], F32, tag="mm", space="PSUM")
                    for kt in range(KT):
                        nc.tensor.matmul(ps[:], lhsT=wq_h[:, kt, 128 * mm_:128 * (mm_ + 1)],
                                         rhs=xT[:, kt, 512 * n:512 * (n + 1)],
                                         start=(kt == 0), stop=(kt == KT - 1))
                    nc.scalar.activation(out=gq[:, m, 2 + 512 * n:2 + 512 * (n + 1)], in_=ps[:],
                                         func=AF.Identity, bias=bq_col_sb[:, m:m + 1], scale=1.0)
            # qT-pass (o-cols of this half) -> DRAM qt_scr[lt*128+p, o]
            for lt in range(NS):
                ps = psmm.tile([128, 512], F32, tag="mm", space="PSUM")
                for kt in range(KT):
                    nc.tensor.matmul(ps[:], lhsT=xT[:, kt, 128 * lt:128 * (lt + 1)],
                                     rhs=wq_h[:, kt, :],
                                     start=(kt == 0), stop=False)
                nc.tensor.matmul(ps[:], lhsT=ones_sb[:],
                                 rhs=bqr_sb[:, 512 * hi:512 * (hi + 1)],
                                 start=False, stop=True)
                qe = qp.tile([128, 512], BF16, tag="qe")
                nc.vector.tensor_copy(out=qe[:], in_=ps[:])
                nc.sync.dma_start(out=bass.AP(
                    tensor=qt_ap.tensor, offset=lt * 128 * C + 512 * hi,
                    ap=[[C, 128], [1, 512]]), in_=qe[:])

        if DBG and b == 0:
            nc.sync.dma_start(out=t['d_gq'].ap(), in_=gq[:])

        # ============== phase B: per group conv -> offsets -> gather -> xs slab ==============
        for g in range(G):
            # conv1 -> h padded (front pad 2)
            h = hp.tile([128, SLG, L + 2], BF16, tag="hpad")
            nc.gpsimd.memset(h[:, :, 0:2], 0.0)
            for mt in range(SLG):
                for n in range(NB):
                    ps = psmm.tile([128, 512], F32, tag="mm", space="PSUM")
                    first = True
                    for tt_ in range(K):
                        for kt2 in range(SLG):
                            nc.tensor.matmul(
                                ps[:],
                                lhsT=w1t_sb[:, kt2, tt_, 128 * mt:128 * (mt + 1)],
                                rhs=gq[:, g * SLG + kt2, 512 * n + tt_: 512 * n + tt_ + 512],
                                start=first, stop=(tt_ == K - 1 and kt2 == SLG - 1))
                            first = False
                    nc.scalar.activation(out=h[:, mt, 2 + 512 * n:2 + 512 * (n + 1)], in_=ps[:],
                                         func=AF.Identity, bias=boff1_sb[:, mt:mt + 1], scale=1.0)

            # conv2 -> off psum [128, NS] (col s holds l' = 128s+p)
            offp = psoff.tile([128, NS], F32, tag="off", space="PSUM")
            for s in range(NS):
                for kt2 in range(SLG):
                    nc.tensor.matmul(offp[:, s:s + 1],
                                     lhsT=h[:, kt2, 128 * s:128 * (s + 1)],
                                     rhs=w2_sb[:, kt2:kt2 + 1],
                                     start=(kt2 == 0), stop=(kt2 == SLG - 1))

            # offsets -> positions -> floor/frac -> int16 idx (i0 | i1)
            tanh_t = sm.tile([128, NS], F32, tag="tanh")
            nc.scalar.activation(out=tanh_t[:], in_=offp[:], func=AF.Tanh,
                                 bias=boff2_sb[:, 0:1], scale=1.0)
            psm = sm.tile([128, NS], F32, tag="psm")
            nc.vector.tensor_scalar(out=psm[:], in0=tanh_t[:], scalar1=5.0 * c.sn,
                                    scalar2=None, op0=OP.mult)
            nc.vector.tensor_tensor(out=psm[:], in0=psm[:], in1=posg_sb[:], op=OP.add)
            if DBG and b == 0:
                nc.sync.dma_start(out=t['d_psm'].ap()[g], in_=psm[:])
                nc.sync.dma_start(out=t['d_h'].ap()[g], in_=h[:])
            i0i = sm.tile([128, NS], I32, tag="i0i")
            nc.vector.tensor_copy(out=i0i[:], in_=psm[:])
            i0f = sm.tile([128, NS], F32, tag="i0f")
            nc.vector.tensor_copy(out=i0f[:], in_=i0i[:])
            w1 = sm.tile([128, NS], F32, tag="w1")
            nc.vector.tensor_tensor(out=w1[:], in0=psm[:], in1=i0f[:], op=OP.subtract)
            nc.vector.tensor_scalar(out=w1[:], in0=w1[:], scalar1=0.5, scalar2=None, op0=OP.add)
            # cast rounding-mode robustness: if w1 >= 1, shift base row up by one
            adj = sm.tile([128, NS], F32, tag="adj")
            nc.vector.tensor_scalar(out=adj[:], in0=w1[:], scalar1=1.0, scalar2=None, op0=OP.is_ge)
            nc.vector.tensor_tensor(out=w1[:], in0=w1[:], in1=adj[:], op=OP.subtract)
            nc.vector.tensor_tensor(out=i0f[:], in0=i0f[:], in1=adj[:], op=OP.add)
            if DBG and b == 0:
                nc.sync.dma_start(out=t['d_w1'].ap()[g], in_=w1[:])
            idxf = sm.tile([128, NS], F32, tag="idxf")
            idx16 = sm.tile([128, 2 * NS], I16, tag="idx16")
            nc.vector.tensor_scalar(out=idxf[:], in0=i0f[:], scalar1=-10.0, scalar2=None, op0=OP.add)
            nc.vector.tensor_copy(out=idx16[:, 0:NS], in_=idxf[:])
            nc.vector.tensor_scalar(out=idxf[:], in0=i0f[:], scalar1=-9.0, scalar2=None, op0=OP.add)
            nc.vector.tensor_copy(out=idx16[:, NS:2 * NS], in_=idxf[:])

            # wrap idx to [16, NIDX/16] layout (replicated over partition groups) via DRAM
            NIDX = 2 * L
            scr = dscr.tile([128, 2 * NS], I16, tag="iscr")
            nc.sync.dma_start(out=scr[:], in_=idx16[:])
            wrapped = sm.tile([128, 16 * NS], I16, tag="wrap")
            scr_ap = scr[:]
            for kk in range(8):
                nc.sync.dma_start(out=wrapped[16 * kk:16 * (kk + 1), :], in_=bass.AP(
                    tensor=scr_ap.tensor, offset=scr_ap.offset,
                    ap=[[2 * NS, 16], [1, 2 * NS], [16 * 2 * NS, 8]]))

            if DBG and b == 0:
                nc.sync.dma_start(out=t['d_idx'].ap()[g], in_=idx16[:])
                nc.sync.dma_start(out=t['d_wrap'].ap()[g], in_=wrapped[:])
            # gather rows (i0 block | i1 block); dma_gather handles <=1024 idxs per call
            g01 = med.tile([128, 2 * NS, GC], BF16, tag="g01")
            CH = min(1024, NIDX)
            for j in range(NIDX // CH):
                nc.gpsimd.dma_gather(
                    g01[:, (CH // 128) * j:(CH // 128) * (j + 1), :],
                    xg[b * LG:(b + 1) * LG, GC * g:GC * (g + 1)],
                    wrapped[:, (CH // 16) * j:(CH // 16) * (j + 1)],
                    CH, CH, GC, elem_step=C)

            if DBG and b == 0:
                nc.sync.dma_start(out=t['d_g01'].ap()[g], in_=g01[:])
            # blend in place: g1 <- (g1 - g0) * w1 ; xsl = g1 + g0
            nc.vector.tensor_tensor(out=g01[:, NS:2 * NS, :], in0=g01[:, NS:2 * NS, :],
                                    in1=g01[:, 0:NS, :], op=OP.subtract)
            for s in range(NS):
                nc.vector.tensor_scalar(out=g01[:, NS + s, :], in0=g01[:, NS + s, :],
                                        scalar1=w1[:, s:s + 1], scalar2=None, op0=OP.mult)
            xsl = med.tile([128, NS, GC], BF16, tag="xsl")
            nc.vector.tensor_tensor(out=xsl[:], in0=g01[:, NS:2 * NS, :],
                                    in1=g01[:, 0:NS, :], op=OP.add)
            xs_ap = t['xs_scr'].ap()
            nc.sync.dma_start(out=bass.AP(
                tensor=xs_ap.tensor, offset=b * L * C + GC * g,
                ap=[[C, 128], [128 * C, NS], [1, GC]]), in_=xsl[:])

        # ============== phase C: xs reload, kT-pass, scores+softmax ==============
        xs = big.tile([128, KT, L], BF16, tag="bigX")
        for kt in range(KT):
            nc.sync.dma_start(
                out=xs[:, kt, :],
                in_=t['xs_scr'].ap()[b * L:(b + 1) * L,
                                     128 * kt:128 * (kt + 1)].rearrange("l c -> c l"))

        kTt = big.tile([128, NS, C], BF16, tag="bigKA")
        for hi in range(NO):
            wk_h = load_whalf('wkT', hi)
            for lt in range(NS):
                ps = psmm.tile([128, 512], F32, tag="mm", space="PSUM")
                for kt in range(KT):
                    nc.tensor.matmul(ps[:], lhsT=xs[:, kt, 128 * lt:128 * (lt + 1)],
                                     rhs=wk_h[:, kt, :],
                                     start=(kt == 0), stop=False)
                nc.tensor.matmul(ps[:], lhsT=ones_sb[:],
                                 rhs=bkr_sb[:, 512 * hi:512 * (hi + 1)],
                                 start=False, stop=True)
                nc.vector.tensor_copy(out=kTt[:, lt, 512 * hi:512 * (hi + 1)], in_=ps[:])

        if DBG and b == 0:
            nc.sync.dma_start(out=t['d_kT'].ap(), in_=kTt[:])
        # scores + softmax + transposed block-diag attn (pairs of heads)
        attnTs = []
        for pr in range(c.P2):
            ps_sc = pssc.tile([128, 64], F32, tag="sc", space="PSUM")
            qsl = qp.tile([128, NS, 128], BF16, tag="qsl")
            nc.sync.dma_start(out=qsl[:], in_=bass.AP(
                tensor=qt_ap.tensor, offset=128 * pr,
                ap=[[C, 128], [128 * C, NS], [1, 128]]))
            for h2 in range(2):
                hh = 2 * pr + h2
                tp = (0, 64 * h2) if h2 else None
                for lt in range(NS):
                    nc.tensor.matmul(ps_sc[64 * h2:64 * (h2 + 1), :],
                                     lhsT=qsl[:, lt, 64 * h2:64 * (h2 + 1)],
                                     rhs=kTt[:, lt, 64 * hh:64 * (hh + 1)],
                                     start=(lt == 0), stop=(lt == NS - 1),
                                     tile_position=tp)
            rmax = sm.tile([128, 1], F32, tag="rmax")
            nc.vector.reduce_max(out=rmax[:], in_=ps_sc[:], axis=mybir.AxisListType.X)
            nb_ = sm.tile([128, 1], F32, tag="nb")
            nc.vector.tensor_scalar(out=nb_[:], in0=rmax[:], scalar1=-scale, scalar2=None, op0=OP.mult)
            expt = sm.tile([128, 64], F32, tag="expt")
            nc.scalar.activation(out=expt[:], in_=ps_sc[:], func=AF.Exp,
                                 bias=nb_[:], scale=scale)
            rsum = sm.tile([128, 1], F32, tag="rsum")
            nc.vector.reduce_sum(out=rsum[:], in_=expt[:], axis=mybir.AxisListType.X)
            rinv = sm.tile([128, 1], F32, tag="rinv")
            nc.vector.reciprocal(out=rinv[:], in_=rsum[:])
            ablk = sm.tile([128, 128], F32, tag="ablk")
            nc.gpsimd.memset(ablk[:], 0.0)
            nc.vector.tensor_scalar(out=ablk[0:64, 0:64], in0=expt[0:64, :],
                                    scalar1=rinv[0:64, :], scalar2=None, op0=OP.mult)
            nc.vector.tensor_scalar(out=ablk[64:128, 64:128], in0=expt[64:128, :],
                                    scalar1=rinv[64:128, :], scalar2=None, op0=OP.mult)
            if DBG and b == 0:
                nc.sync.dma_start(out=t['d_ablk'].ap()[pr], in_=ablk[:])
            trp = pstr.tile([128, 128], F32, tag="tr", space="PSUM")
            nc.tensor.transpose(trp[:], ablk[:], ident[:])
            aT = sm.tile([128, 128], BF16, tag=f"aT{pr}")
            nc.vector.tensor_copy(out=aT[:], in_=trp[:])
            attnTs.append(aT)

        # ============== phase D: v-pass + attn@v ==============
        v = big.tile([128, KT, L], BF16, tag="bigGV")
        for hi in range(NO):
            wv_h = load_whalf('wvT', hi)
            for mm_ in range(MH):
                m = hi * MH + mm_
                for n in range(NB):
                    ps = psmm.tile([128, 512], F32, tag="mm", space="PSUM")
                    for kt in range(KT):
                        nc.tensor.matmul(ps[:], lhsT=wv_h[:, kt, 128 * mm_:128 * (mm_ + 1)],
                                         rhs=xs[:, kt, 512 * n:512 * (n + 1)],
                                         start=(kt == 0), stop=(kt == KT - 1))
                    rt = rp.tile([128, 512], BF16, tag="rt")
                    nc.sync.dma_start(out=rt[:], in_=t['rtab'].ap()[128 * m:128 * (m + 1),
                                                                    512 * n:512 * (n + 1)])
                    nc.vector.tensor_tensor(out=v[:, m, 512 * n:512 * (n + 1)],
                                            in0=ps[:], in1=rt[:], op=OP.add)

        if DBG and b == 0:
            nc.sync.dma_start(out=t['d_v'].ap(), in_=v[:])
        # attn @ v -> ao^T
        ao = big.tile([128, KT, L], BF16, tag="bigKA")
        for pr in range(c.P2):
            for n in range(NB):
                ps = psmm.tile([128, 512], F32, tag="mm", space="PSUM")
                nc.tensor.matmul(ps[:], lhsT=attnTs[pr][:],
                                 rhs=v[:, pr, 512 * n:512 * (n + 1)],
                                 start=True, stop=True)
                nc.vector.tensor_copy(out=ao[:, pr, 512 * n:512 * (n + 1)], in_=ps[:])

        if DBG and b == 0:
            nc.sync.dma_start(out=t['d_ao'].ap(), in_=ao[:])
        # ============== phase E: y^T = Wout @ ao^T + bout -> out ==============
        out_ap = t['out2d'].ap()
        for hi in range(NO):
            wo_h = load_whalf('woutT', hi)
            for mm_ in range(MH):
                m = hi * MH + mm_
                for n in range(NB):
                    ps = psmm.tile([128, 512], F32, tag="mm", space="PSUM")
                    for kt in range(KT):
                        nc.tensor.matmul(ps[:], lhsT=wo_h[:, kt, 128 * mm_:128 * (mm_ + 1)],
                                         rhs=ao[:, kt, 512 * n:512 * (n + 1)],
                                         start=(kt == 0), stop=(kt == KT - 1))
                    yt = yp.tile([128, 512], F32, tag="yt")
                    nc.scalar.activation(out=yt[:], in_=ps[:], func=AF.Identity,
                                         bias=bout_sb[:, m:m + 1], scale=1.0)
                    nc.sync.dma_start(out=bass.AP(
                        tensor=out_ap.tensor, offset=(b * L + 512 * n) * C + 128 * m,
                        ap=[[1, 128], [C, 512]]), in_=yt[:])


def make_nc(cfg):
    nc = bacc.Bacc("TRN2", target_bir_lowering=False, debug=False)
    t = declare(nc, cfg)
    from contextlib import ExitStack
    with tile.TileContext(nc) as tc:
        with ExitStack() as ctx:
            build(tc, t, cfg, ctx)
    nc.compile()
    return nc


def host_prep_shared(inputs, cfg):
    c = cfg
    Wq, Wk, Wv, Wout = inputs['Wq'], inputs['Wk'], inputs['Wv'], inputs['Wout']
    sh = {
        'wqT': np.ascontiguousarray(Wq.T).astype(BF),
        'wkT': np.ascontiguousarray(Wk.T).astype(BF),
        'wvT': np.ascontiguousarray(Wv.T).astype(BF),
        'woutT': np.ascontiguousarray(Wout.T).astype(BF),
        'w1t': np.ascontiguousarray(np.transpose(inputs['Woff1'], (1, 2, 0))).astype(BF),
        'w2c': np.ascontiguousarray(inputs['Woff2'][0, :, 0:1]).astype(BF),
        'bq_col': inputs['bq'][:, None].astype(np.float32),
        'bq_row': inputs['bq'][None, :].astype(BF),
        'bk_row': inputs['bk'][None, :].astype(BF),
        'boff1_col': inputs['boff1'][:, None].astype(np.float32),
        'boff2c': inputs['boff2'][:, None].astype(np.float32),
        'bout_col': inputs['bout'][:, None].astype(np.float32),
        'rtab': (inputs['bv'][:, None] + inputs['rpb_table'][0]).astype(BF),
        'posg': ((np.arange(128)[:, None] + 128 * np.arange(c.NS)[None, :]) * c.sn
                 + 15.0).astype(np.float32),
    }
    return sh


def host_prep_core(x_shard, cfg):
    c = cfg
    xgp = np.zeros((c.B_SH, c.L + c.GR, c.C), np.float32)
    xgp[:, c.GO:c.GO + c.L] = x_shard
    return {'xgbf': xgp.reshape(c.B_SH * (c.L + c.GR), c.C).astype(BF)}


# ----------------------------------------------------------------------------
# Public entry point
# ----------------------------------------------------------------------------
_N_CORES = 8
_B, _L, _C, _H, _G, _K = 16, 2048, 1024, 16, 4, 5
_CACHE = {}


def _get_nc(cfg):
    if 'nc' not in _CACHE:
        _CACHE['nc'] = make_nc(cfg)
    return _CACHE['nc']


def kernel(**inputs):
    inputs = {k: np.asarray(v) for k, v in inputs.items()}
    cfg = Cfg(B_SH=_B // _N_CORES, L=_L, C=_C, H=_H, G=_G, K=_K)
    nc = _get_nc(cfg)
    sh = host_prep_shared(inputs, cfg)
    in_maps = [
        {**sh, **host_prep_core(inputs['x'][c * cfg.B_SH:(c + 1) * cfg.B_SH], cfg)}
        for c in range(_N_CORES)
    ]
    from concourse.bass_utils import run_bass_kernel_spmd
    res = run_bass_kernel_spmd(nc, in_maps, core_ids=list(range(_N_CORES)))
    out = np.concatenate(
        [res.results[c]["out2d"].reshape(cfg.B_SH, _L, _C) for c in range(_N_CORES)],
        axis=0)
    return out.astype(np.float32)

